# revision 16
# baseline (speedup 1.0000x reference)
"""CrossAssetGNN forward on 8 Trainium2 cores, data-parallel over batch.

bf16 rewrite of the fp32 baseline (653us). Key structural changes:
- All matmuls/transposes run in bf16 (1 cyc/row vs 4 for fp32, single
  hardware pass instead of two, cheaper LDWEIGHTS).
- The node-embedding matmul is folded into conv1 on the host:
  conv1(emb(x)) = sum_k (C1k @ W_emb) @ x_shift_k, so the kernel starts
  straight from the 15-timestep x slice.
- Edge-weight MLP: R blocks are built src-major ((v,g,j) col order) by
  DVE+GpSimd halves (outer-sum via stride-0 broadcast APs), relu on the
  Act engine, then 520 bf16 matmuls (R-block stationary, w2 streaming)
  produce ew^T[src, (g,dst)] directly; the 2 aux-j rows collapse to four
  [1,260] row-matmuls plus 2 partition-shifting DMAs.
- GAT layers work on the TRANSPOSED attention matrix: alpha^T is built
  by PE rank-2 matmuls, the exp/count chain multiplies by C^T (host
  upload), so P^T (the aggregation stationary) appears with NO on-device
  transposes of P; row sums come from a ones-vector matmul; only the
  nf node-major -> feature-major transpose remains (2 per graph).
- Per-asset heads: 2-asset-packed stationaries with zero-padded
  block-diagonal W2, junk quadrants killed by the zero blocks.
"""
import json
import sys

sys.path.insert(0, "/opt/trn_rl_repo")

import numpy as np
import ml_dtypes
from contextlib import ExitStack

import concourse.bass as bass
import concourse.tile as tile
from concourse import masks, mybir
from concourse.bass_utils import run_bass_kernel_spmd

f32 = mybir.dt.float32
bf16 = mybir.dt.bfloat16
AF = mybir.ActivationFunctionType
OP = mybir.AluOpType
BF = ml_dtypes.bfloat16

B, A, AUX, T, DIN, H, ODIM = 32, 128, 2, 128, 64, 128, 3
N = A + AUX            # 130
NC_CORES = 8
NB = B // NC_CORES     # 4 graphs per core
W = 15                 # receptive field of the three causal convs
BN_EPS = 1e-5
NCOL = NB * N          # 520 node columns per core


# ---- walrus workaround: max 1 sync-wait command per instruction ------------
def _apply_sync_split_patch():
    if getattr(bass.Bass, "_sync_split_patched", False):
        return
    orig = bass.Bass.to_json_bytes

    def to_json_bytes(self, *a, **kw):
        m = json.loads(orig(self, *a, **kw))
        for f in m.get("functions", []):
            for blk in f.get("blocks", []):
                new = []
                for inst in blk.get("instructions", []):
                    si = inst.get("sync_info")
                    if (si and si.get("on_wait") and len(si["on_wait"]) > 1
                            and inst.get("engine") in
                            {"PE", "DVE", "Activation", "SP", "Pool"}):
                        waits = si["on_wait"]
                        for k, w in enumerate(waits[:-1]):
                            new.append({"engine": inst["engine"], "ins": [],
                                        "outs": [],
                                        "name": f"{inst['name']}-sw{k}",
                                        "opcode": "NoOp",
                                        "sync_info": {"on_update": [],
                                                      "on_wait": [w]}})
                        si["on_wait"] = waits[-1:]
                    new.append(inst)
                blk["instructions"] = new
        return json.dumps(m).encode()

    bass.Bass.to_json_bytes = to_json_bytes
    bass.Bass._sync_split_patched = True


def _ap(t, offset_elems, dims):
    """AP over tile t: partition dim + given free [step, count] dims."""
    return bass.AP(tensor=t.tensor, offset=t.offset + offset_elems,
                   ap=[list(t.ap[0])] + [list(d) for d in dims])


def _papp(t, p0, p1, offset_elems, dims):
    """AP with partition slice [p0:p1] + free dims."""
    base = t[p0:p1, 0:1]
    return bass.AP(tensor=base.tensor, offset=base.offset + offset_elems,
                   ap=[list(base.ap[0])] + [list(d) for d in dims])


def _chunks(total, step):
    return [(s, min(step, total - s)) for s in range(0, total, step)]


def build_program():
    nc = bass.Bass("TRN2", target_bir_lowering=False, num_devices=NC_CORES)

    din = {}

    def d_in(name, shape, dt=bf16):
        din[name] = nc.dram_tensor(name, list(shape), dt, kind="ExternalInput")
        return din[name]

    d_in("xt", [DIN, NCOL * W])
    d_in("c1wT", [DIN, 3 * H])          # conv1 taps folded with W_emb
    d_in("cwT", [H, 6 * H])             # conv2/conv3 taps
    d_in("sc_all", [H, 3], f32)
    d_in("bi_all", [H, 3], f32)
    d_in("W1aT", [H, H]); d_in("W1bT", [H, H])
    d_in("b1f", [H, 1], f32)
    d_in("w2b", [H, 1])
    d_in("b2ew", [1, 1], f32)
    d_in("CAT", [128, N]); d_in("CBT", [2, N])     # C^T rows
    d_in("gWT", [H, 3 * H])
    d_in("asrcb", [H, 3]); d_in("adstb", [H, 3])
    d_in("hW1T", [H, A * 64])
    d_in("b1exp3", [128, A * NB], f32)
    d_in("W2blk", [H, (A // 2) * 2 * ODIM])
    d_in("b2exp", [ODIM, A * NB], f32)

    o_logits = nc.dram_tensor("logits", [ODIM, A * NB], f32,
                              kind="ExternalOutput")
    o_probs = nc.dram_tensor("probs", [128, NB * ODIM], f32,
                             kind="ExternalOutput")

    with tile.TileContext(nc) as tc:
        with ExitStack() as top:
            const = top.enter_context(tc.tile_pool(name="const", bufs=1))
            persist = top.enter_context(tc.tile_pool(name="persist", bufs=1))
            front = top.enter_context(tc.tile_pool(name="front", bufs=1))

            # upload order: xt(g0) and the conv weights gate the pipeline
            # head, so they go before xt(g1..3) on the queues
            xT = front.tile([DIN, NCOL * W], bf16)

            def load(name, shape, dt=bf16):
                t = const.tile(list(shape), dt, name=f"c_{name}",
                               tag=f"c_{name}")
                nc.sync.dma_start(out=t, in_=din[name][:, :])
                return t

            nc.sync.dma_start(out=xT[:, 0:N * W], in_=din["xt"][:, 0:N * W])
            c1wT = load("c1wT", [DIN, 3 * H])
            sc_all = load("sc_all", [H, 3], f32)
            bi_all = load("bi_all", [H, 3], f32)
            cwT = load("cwT", [H, 6 * H])
            for g in range(1, NB):
                nc.sync.dma_start(
                    out=xT[:, g * N * W:(g + 1) * N * W],
                    in_=din["xt"][:, g * N * W:(g + 1) * N * W])
            W1aT = load("W1aT", [H, H]); W1bT = load("W1bT", [H, H])
            b1f = load("b1f", [H, 1], f32)
            w2b = load("w2b", [H, 1])
            CAT = load("CAT", [128, N]); CBT = load("CBT", [2, N])
            gWT = load("gWT", [H, 3 * H])
            asrcb = load("asrcb", [H, 3]); adstb = load("adstb", [H, 3])

            b2ap = din["b2ew"][:, :]
            b2col = const.tile([128, 1], f32)
            nc.sync.dma_start(out=b2col, in_=bass.AP(
                tensor=b2ap.tensor, offset=b2ap.offset, ap=[[0, 128], [1, 1]]))

            identb = const.tile([128, 128], bf16)
            masks.make_identity(nc, identb[:, :])
            identf = const.tile([128, 128], f32)
            masks.make_identity(nc, identf[:, :])
            alpha02 = const.tile([128, 1], f32)
            nc.vector.memset(alpha02[:, :], 0.2)
            ones128b = const.tile([128, 1], bf16)
            nc.vector.memset(ones128b[:, :], 1.0)
            ones_row = const.tile([1, NCOL], bf16)
            nc.vector.memset(ones_row[:, :], 1.0)

            feats = persist.tile([H, NCOL], bf16, name="feats")
            Ut = persist.tile([H, NCOL], bf16, name="Ut")
            Vt = persist.tile([H, NCOL], bf16, name="Vt")
            ewT_sb = persist.tile([128, NCOL], bf16, name="ewT")
            ewT2 = persist.tile([2, NCOL], bf16, name="ewT2")
            nfT_a = persist.tile([H, NCOL], bf16, name="nfT_a")
            nfT_b = persist.tile([H, NCOL], bf16, name="nfT_b")

            # ------- stages A+B fused: per-graph conv cascade + edge MLP ----
            # The conv chain is graph-local (conv2(g) reads only l1(g), etc),
            # so graph g's edge-MLP elementwise work (DVE/GpSimd) runs UNDER
            # the PE convs of graphs g+1.. .  R_g cols = vlp*256 + j*2 + t
            # (v = 2*vlp+t, j = src 0..127): the t in {0,1} inner pair gives
            # every DVE operand a stride-1 last AP dim -> 2x DVE mode
            # (0.53ns/elem) vs 1x for the plain broadcast outer-sum.
            RW = 65 * 256              # 16640 R cols per graph
            zero1 = const.tile([128, 1], bf16)
            nc.vector.memset(zero1[:, :], 0.0)
            with ExitStack() as sAB:
                psA = sAB.enter_context(
                    tc.tile_pool(name="psA", bufs=3, space="PSUM"))
                psU = sAB.enter_context(
                    tc.tile_pool(name="psU", bufs=1, space="PSUM"))
                psE = sAB.enter_context(
                    tc.tile_pool(name="psE", bufs=1, space="PSUM"))
                ewk = sAB.enter_context(tc.tile_pool(name="ewk", bufs=2))

                ewTPS4 = psE.tile([128, NB * N], f32, name="ewTPS4",
                                  tag="ewTPS4")


                # per-graph split: GpSimd takes the last nv_gps dsts as a
                # v-major add region (2-dim broadcast APs; Pool rejects
                # TT-max and 3-dim APs); DVE pair-interleaves the rest.
                # relu mostly on Act (free after gelu); DVE TS-relu (4x)
                # keeps a fixed share. in0 offsets stay != 8 (mod 16) elems
                # (that congruence class falls off the 2x DVE fast path).
                nv_gps = 18
                nblk = (N - nv_gps) // 2   # 56 DVE vlp blocks
                rgps = nblk * 256
                relu_dve = 16

                # conv1+conv2 batched across graphs (cross-graph chunk
                # pipelining keeps PE ahead of the Act gelu evictions)
                l1 = ewk.tile([H, NCOL * 13], bf16, name="l1", tag="R3",
                              bufs=1)
                l2 = front.tile([H, NCOL * 9], bf16)
                convs = [(xT, c1wT, 0, W, 13, 1, 0, l1, 39),
                         (l1, cwT, 0, 13, 9, 2, 1, l2, 56)]
                for src_, taps, tap0, in_len, out_len, dil, li, dst, bpc in convs:
                    sv = src_.rearrange("p (blk t) -> p blk t", t=in_len)
                    for b0, nb in _chunks(NCOL, bpc):
                        pe = psA.tile([128, 512], f32, tag="pe")
                        w_cols = nb * out_len
                        for k in range(3):
                            rhs = sv[:, b0:b0 + nb, k * dil:k * dil + out_len]
                            nc.tensor.matmul(
                                pe[:, :w_cols],
                                lhsT=taps[:, (tap0 + k) * H:(tap0 + k + 1) * H],
                                rhs=rhs, start=(k == 0), stop=(k == 2))
                        nc.scalar.activation(
                            dst[:, b0 * out_len:b0 * out_len + w_cols],
                            pe[:, :w_cols], AF.Gelu,
                            bias=bi_all[:, li:li + 1], scale=sc_all[:, li:li + 1])

                l2v = l2.rearrange("p (blk t) -> p blk t", t=9)
                Rt = []
                for g in range(NB):
                    gb = g * N
                    # conv3 for graph g
                    pe3 = psA.tile([128, 512], f32, tag="pe", name="pe3")
                    for k in range(3):
                        nc.tensor.matmul(
                            pe3[:, :N],
                            lhsT=cwT[:, (3 + k) * H:(4 + k) * H],
                            rhs=l2v[:, gb:gb + N, k * 4:k * 4 + 1],
                            start=(k == 0), stop=(k == 2))
                    nc.scalar.activation(feats[:, gb:gb + N], pe3[:, :N],
                                         AF.Gelu, bias=bi_all[:, 2:3],
                                         scale=sc_all[:, 2:3])

                    # U/V rows for graph g
                    pu = psU.tile([128, N], f32, tag="uv")
                    nc.tensor.matmul(pu[:, :], lhsT=W1aT[:, :],
                                     rhs=feats[:, gb:gb + N], start=True,
                                     stop=True)
                    nc.vector.tensor_copy(Ut[:, gb:gb + N], pu[:, :])
                    pv = psU.tile([128, N], f32, tag="uv")
                    nc.tensor.matmul(pv[:, :], lhsT=W1bT[:, :],
                                     rhs=feats[:, gb:gb + N], start=True,
                                     stop=True)
                    nc.scalar.activation(Vt[:, gb:gb + N], pv[:, :],
                                         AF.Identity, bias=b1f[:, :])

                    # pair-interleaved outer-sum + relu (split across engines)
                    R = ewk.tile([128, RW], bf16, name=f"R{g}",
                                 tag=f"R{g}", bufs=1)
                    Rt.append(R)
                    Udup = ewk.tile([128, 256], bf16, tag="Udup", bufs=2)
                    nc.vector.tensor_copy(
                        _ap(Udup, 0, [[2, 128], [1, 2]]),
                        _ap(Ut, gb, [[1, 128], [0, 2]]))
                    nc.gpsimd.tensor_tensor(
                        out=_ap(R, rgps, [[128, nv_gps], [1, 128]]),
                        in0=_ap(Vt, gb + N - nv_gps, [[1, nv_gps], [0, 128]]),
                        in1=_ap(Ut, gb, [[0, nv_gps], [1, 128]]),
                        op=OP.add)
                    for h0, hn in ((0, 32), (32, nblk - 32)):
                        nc.vector.tensor_tensor(
                            out=_ap(R, h0 * 256, [[256, hn], [1, 256]]),
                            in0=_ap(Vt, gb + 2 * h0, [[2, hn], [0, 128],
                                                      [1, 2]]),
                            in1=_ap(Udup, 0, [[0, hn], [1, 256]]),
                            op=OP.add)
                    nc.vector.tensor_scalar_max(
                        _ap(R, 0, [[256, relu_dve], [1, 256]]),
                        _ap(R, 0, [[256, relu_dve], [1, 256]]), 0.0)
                    for r0, rn in _chunks(nblk - relu_dve, 20):
                        nc.scalar.activation(
                            R[:, (relu_dve + r0) * 256:
                              (relu_dve + r0 + rn) * 256],
                            R[:, (relu_dve + r0) * 256:
                              (relu_dve + r0 + rn) * 256], AF.Relu)
                    nc.scalar.activation(R[:, rgps:rgps + nv_gps * 128],
                                         R[:, rgps:rgps + nv_gps * 128],
                                         AF.Relu)

                # reduce (strided 1-col matmuls, LDWEIGHTS-free) + sigmoid,
                # deferred so the PE FIFO never head-of-line blocks on relu
                for g in range(NB):
                    R = Rt[g]
                    for vlp in range(nblk):
                        for t in range(2):
                            v = 2 * vlp + t
                            nc.tensor.matmul(
                                ewTPS4[:, g * N + v:g * N + v + 1],
                                lhsT=_ap(R, vlp * 256 + t, [[2, 128]]),
                                rhs=w2b[:, :], start=True, stop=True)
                    for vi in range(nv_gps):
                        v = N - nv_gps + vi
                        nc.tensor.matmul(
                            ewTPS4[:, g * N + v:g * N + v + 1],
                            lhsT=_ap(R, rgps + vi * 128, [[1, 128]]),
                            rhs=w2b[:, :], start=True, stop=True)
                    nc.scalar.activation(
                        ewT_sb[:, g * N:(g + 1) * N],
                        ewTPS4[:, g * N:(g + 1) * N], AF.Sigmoid,
                        bias=b2col[:, :])



                # aux-j rows (src 128..129): cols (j2, g, v)
                Raux = ewk.tile([128, 2 * NCOL], bf16, tag="Raux")
                for j2 in range(2):
                    nc.vector.tensor_tensor(
                        out=_ap(Raux, j2 * NCOL, [[N, NB], [1, N]]),
                        in0=_ap(Ut, 128 + j2, [[N, NB], [0, N]]),
                        in1=_ap(Vt, 0, [[N, NB], [1, N]]),
                        op=OP.add)
                nc.scalar.activation(Raux[:, :], Raux[:, :], AF.Relu)
                ewstage = ewk.tile([1, 2 * NCOL], bf16, tag="ewstage")
                for c in range(4):
                    pax = psU.tile([1, 260], f32, tag="aux")
                    nc.tensor.matmul(pax[0:1, :],
                                     lhsT=w2b[:, :],
                                     rhs=Raux[:, c * 260:(c + 1) * 260],
                                     start=True, stop=True)
                    nc.scalar.activation(ewstage[0:1, c * 260:(c + 1) * 260],
                                         pax[0:1, :], AF.Sigmoid,
                                         bias=b2col[0:1, :])
                nc.sync.dma_start(out=ewT2[0:1, :], in_=ewstage[0:1, 0:NCOL])
                nc.sync.dma_start(out=ewT2[1:2, :],
                                  in_=ewstage[0:1, NCOL:2 * NCOL])


            # late-load the bulky head weights (needed only in stage D)
            hW1T = load("hW1T", [H, A * 64])
            b1exp3 = load("b1exp3", [128, A * NB], f32)
            W2blk = load("W2blk", [H, (A // 2) * 2 * ODIM])
            b2exp = load("b2exp", [ODIM, A * NB], f32)

            # ---------------- stage C: 3 GAT layers (transposed P) ----------
            nfT_cur = feats
            with ExitStack() as sC:
                gw = sC.enter_context(tc.tile_pool(name="gw", bufs=2))
                gps = sC.enter_context(
                    tc.tile_pool(name="gps", bufs=1, space="PSUM"))

                # AS2 rows: (as, ones); AD2 rows: (ones, ad) — const rows
                # preset once, per-layer rows written below
                AS2 = persist.tile([2, NCOL], bf16, name="AS2")
                nc.sync.dma_start(out=AS2[1:2, :], in_=ones_row[0:1, :])
                AD2 = persist.tile([2, NCOL], bf16, name="AD2")
                nc.vector.tensor_copy(AD2[0:1, :], ones_row[0:1, :])

                for li in range(3):
                    gW = gWT[:, li * H:(li + 1) * H]
                    nfT_next = nfT_a if li % 2 == 0 else nfT_b

                    # as/ad rows directly from nfT via host-folded W^T a
                    ad_stage = gw.tile([1, NCOL], bf16, tag="ad_stage")
                    for s, ln in _chunks(NCOL, 512):
                        pr = gps.tile([128, 512], f32, tag="big", bufs=1)
                        nc.tensor.matmul(pr[0:1, :ln],
                                         lhsT=asrcb[:, li:li + 1],
                                         rhs=nfT_cur[:, s:s + ln],
                                         start=True, stop=True)
                        nc.vector.tensor_copy(AS2[0:1, s:s + ln],
                                              pr[0:1, :ln])
                        pr2 = gps.tile([128, 512], f32, tag="big", bufs=1)
                        nc.tensor.matmul(pr2[0:1, :ln],
                                         lhsT=adstb[:, li:li + 1],
                                         rhs=nfT_cur[:, s:s + ln],
                                         start=True, stop=True)
                        nc.vector.tensor_copy(ad_stage[0:1, s:s + ln],
                                              pr2[0:1, :ln])
                    nc.sync.dma_start(out=AD2[1:2, :], in_=ad_stage[0:1, :])

                    tT = gw.tile([128, NCOL], bf16, tag="tT")
                    tT2 = gw.tile([2, NCOL], bf16, tag="tT2")
                    for gp in range(2):
                        pac = gps.tile([128, 264], f32, tag="pa", bufs=1)
                        pac2 = gps.tile([2, 264], f32, tag="pa2", bufs=1)
                        for k in range(2):
                            g = gp * 2 + k
                            nc.tensor.matmul(pac[:, k * N:(k + 1) * N],
                                             lhsT=AS2[:, g * N:g * N + 128],
                                             rhs=AD2[:, g * N:(g + 1) * N],
                                             start=True, stop=True)
                            nc.tensor.matmul(pac2[0:2, k * N:(k + 1) * N],
                                             lhsT=AS2[:, g * N + 128:(g + 1) * N],
                                             rhs=AD2[:, g * N:(g + 1) * N],
                                             start=True, stop=True)
                        nc.scalar.activation(tT[:, gp * 2 * N:(gp + 1) * 2 * N],
                                             pac[:, 0:2 * N], AF.Prelu,
                                             alpha=alpha02[:, :])
                        nc.scalar.activation(tT2[0:2, gp * 2 * N:(gp + 1) * 2 * N],
                                             pac2[0:2, 0:2 * N], AF.Prelu,
                                             alpha=alpha02[0:2, :])

                    zT = gw.tile([128, NCOL], bf16, tag="zT")
                    nc.vector.tensor_tensor(out=zT[:, :], in0=tT[:, :],
                                            in1=ewT_sb[:, :], op=OP.mult)
                    zT2 = gw.tile([2, NCOL], bf16, tag="zT2")
                    nc.vector.tensor_tensor(out=zT2[:, :], in0=tT2[:, :],
                                            in1=ewT2[:, :], op=OP.mult)
                    eT = gw.tile([128, NCOL], bf16, tag="eT")
                    nc.scalar.activation(eT[:, :], zT[:, :], AF.Exp)
                    eT2 = gw.tile([2, NCOL], bf16, tag="eT2")
                    nc.scalar.activation(eT2[:, :], zT2[:, :], AF.Exp)
                    PT = gw.tile([128, NCOL], bf16, tag="PT")
                    nc.vector.tensor_tensor(
                        out=PT[:, :], in0=eT[:, :],
                        in1=_ap(CAT, 0, [[0, NB], [1, N]]), op=OP.mult)
                    PT2 = gw.tile([2, NCOL], bf16, tag="PT2")
                    nc.vector.tensor_tensor(
                        out=PT2[:, :], in0=eT2[:, :],
                        in1=_ap(CBT, 0, [[0, NB], [1, N]]), op=OP.mult)

                    # per-dst row sums as psum columns (no transposes needed)
                    sumsPS = gps.tile([128, 8], f32, tag="sums", bufs=1)
                    for g in range(NB):
                        nc.tensor.matmul(sumsPS[:, g:g + 1],
                                         lhsT=PT[:, g * N:g * N + 128],
                                         rhs=ones128b[:, :],
                                         start=True, stop=False)
                        nc.tensor.matmul(sumsPS[:, g:g + 1],
                                         lhsT=PT2[:, g * N:g * N + 128],
                                         rhs=ones128b[0:2, :],
                                         start=False, stop=True)
                        if li < 2:
                            nc.tensor.matmul(
                                sumsPS[0:2, 4 + g:5 + g],
                                lhsT=PT[:, g * N + 128:(g + 1) * N],
                                rhs=ones128b[:, :], start=True, stop=False)
                            nc.tensor.matmul(
                                sumsPS[0:2, 4 + g:5 + g],
                                lhsT=PT2[:, g * N + 128:(g + 1) * N],
                                rhs=ones128b[0:2, :], start=False, stop=True)
                    rAe = gw.tile([128, NB], f32, tag="rAe")
                    nc.vector.tensor_scalar_add(rAe[:, :], sumsPS[:, 0:4],
                                                1e-8)
                    rA = gw.tile([128, NB], f32, tag="rA")
                    nc.vector.reciprocal(rA[:, :], rAe[:, :])
                    rexp = gw.tile([128, NB * H], bf16, tag="rexp")
                    nc.vector.tensor_copy(rexp[:, :],
                                          _ap(rA, 0, [[1, NB], [0, H]]))
                    if li < 2:
                        rBe = gw.tile([2, NB], f32, tag="rBe")
                        nc.vector.tensor_scalar_add(rBe[:, :],
                                                    sumsPS[0:2, 4:8], 1e-8)
                        rB = gw.tile([2, NB], f32, tag="rB")
                        nc.vector.reciprocal(rB[:, :], rBe[:, :])
                        rexp2 = gw.tile([2, NB * H], bf16, tag="rexp2")
                        nc.vector.tensor_copy(rexp2[:, :],
                                              _ap(rB, 0, [[1, NB], [0, H]]))

                    poPS = gps.tile([128, NB * H], f32, tag="po", bufs=1)
                    if li < 2:
                        poPS2 = gps.tile([2, NB * H], f32, tag="po2", bufs=1)
                    for g in range(NB):
                        sq = gps.tile([128, 256], f32, tag="sq", bufs=1)
                        nc.tensor.matmul(sq[:, 0:H],
                                         lhsT=nfT_cur[:, g * N:g * N + 128],
                                         rhs=gW, start=True, stop=True)
                        hpA = gw.tile([128, H], bf16, tag="hpA")
                        nc.vector.tensor_copy(hpA[:, :], sq[:, 0:H])
                        nc.tensor.matmul(sq[0:2, H:2 * H],
                                         lhsT=nfT_cur[:, g * N + 128:(g + 1) * N],
                                         rhs=gW, start=True, stop=True)
                        hpB = gw.tile([2, H], bf16, tag="hpB")
                        nc.vector.tensor_copy(hpB[:, :], sq[0:2, H:2 * H])

                        nc.tensor.matmul(poPS[:, g * H:(g + 1) * H],
                                         lhsT=PT[:, g * N:g * N + 128],
                                         rhs=hpA[:, :], start=True, stop=False)
                        nc.tensor.matmul(poPS[:, g * H:(g + 1) * H],
                                         lhsT=PT2[:, g * N:g * N + 128],
                                         rhs=hpB[:, :], start=False, stop=True)
                        if li < 2:
                            nc.tensor.matmul(
                                poPS2[0:2, g * H:(g + 1) * H],
                                lhsT=PT[:, g * N + 128:(g + 1) * N],
                                rhs=hpA[:, :], start=True, stop=False)
                            nc.tensor.matmul(
                                poPS2[0:2, g * H:(g + 1) * H],
                                lhsT=PT2[:, g * N + 128:(g + 1) * N],
                                rhs=hpB[:, :], start=False, stop=True)

                    # batched elu over all 4 graphs: elu(po*r) with r>0
                    pos_all = gw.tile([128, NB * H], bf16, tag="pos_all")
                    nc.scalar.activation(pos_all[:, :], poPS[:, :], AF.Relu)
                    posr = gw.tile([128, NB * H], bf16, tag="posr")
                    nc.vector.tensor_tensor(out=posr[:, :], in0=pos_all[:, :],
                                            in1=rexp[:, :], op=OP.mult)
                    m_all = gw.tile([128, NB * H], bf16, tag="m_all")
                    nc.vector.tensor_scalar_min(m_all[:, :], poPS[:, :], 0.0)
                    mr = gw.tile([128, NB * H], bf16, tag="mr")
                    nc.vector.tensor_tensor(out=mr[:, :], in0=m_all[:, :],
                                            in1=rexp[:, :], op=OP.mult)
                    exm = gw.tile([128, NB * H], bf16, tag="exm")
                    nc.scalar.activation(exm[:, :], mr[:, :], AF.Exp)
                    nf_nm = gw.tile([128, NB * H], bf16, tag="nf_nm")
                    nc.vector.scalar_tensor_tensor(
                        out=nf_nm[:, :], in0=exm[:, :], scalar=1.0,
                        in1=posr[:, :], op0=OP.subtract, op1=OP.add)
                    if li < 2:
                        pos2 = gw.tile([2, NB * H], bf16, tag="pos2")
                        nc.scalar.activation(pos2[:, :], poPS2[:, :], AF.Relu)
                        posr2 = gw.tile([2, NB * H], bf16, tag="posr2")
                        nc.vector.tensor_tensor(out=posr2[:, :],
                                                in0=pos2[:, :],
                                                in1=rexp2[:, :], op=OP.mult)
                        m2 = gw.tile([2, NB * H], bf16, tag="m2")
                        nc.vector.tensor_scalar_min(m2[:, :], poPS2[:, :], 0.0)
                        mr2 = gw.tile([2, NB * H], bf16, tag="mr2")
                        nc.vector.tensor_tensor(out=mr2[:, :], in0=m2[:, :],
                                                in1=rexp2[:, :], op=OP.mult)
                        exm2 = gw.tile([2, NB * H], bf16, tag="exm2")
                        nc.scalar.activation(exm2[:, :], mr2[:, :], AF.Exp)
                        nf_nm2 = gw.tile([2, NB * H], bf16, tag="nf_nm2")
                        nc.vector.scalar_tensor_tensor(
                            out=nf_nm2[:, :], in0=exm2[:, :], scalar=1.0,
                            in1=posr2[:, :], op0=OP.subtract, op1=OP.add)

                    for g in range(NB):
                        ptb = gps.tile([128, 130], bf16, tag="tb", bufs=1)
                        nc.tensor.transpose(ptb[:, 0:128],
                                            nf_nm[:, g * H:(g + 1) * H],
                                            identb[:, :])
                        nc.vector.tensor_copy(nfT_next[:, g * N:g * N + 128],
                                              ptb[:, 0:128])
                        if li < 2:
                            nc.tensor.transpose(ptb[:, 128:130],
                                                nf_nm2[:, g * H:(g + 1) * H],
                                                identb[0:2, 0:2])
                            nc.vector.tensor_copy(
                                nfT_next[:, g * N + 128:(g + 1) * N],
                                ptb[:, 128:130])
                    nfT_cur = nfT_next

            # ---------------- stage D: packed per-asset heads + softmax -----
            with ExitStack() as sD:
                hw = sD.enter_context(tc.tile_pool(name="hw", bufs=1))
                hps = sD.enter_context(
                    tc.tile_pool(name="hps", bufs=1, space="PSUM"))

                hid_ps = hps.tile([128, A * NB], f32, tag="hid")
                for p in range(A // 2):
                    nc.tensor.matmul(
                        hid_ps[:, p * 8:(p + 1) * 8],
                        lhsT=hW1T[:, p * 128:(p + 1) * 128],
                        rhs=_ap(nfT_cur, 2 * p, [[1, 2], [N, NB]]),
                        start=True, stop=True)
                hid_t = hw.tile([128, A * NB], bf16, tag="hid_t")
                nc.vector.tensor_tensor(out=hid_t[:, :], in0=hid_ps[:, :],
                                        in1=b1exp3[:, :], op=OP.add)
                hid3 = hw.tile([128, A * NB], bf16, tag="hid3")
                nc.scalar.activation(hid3[:, :], hid_t[:, :], AF.Relu)

                log_ps = hps.tile([2 * ODIM, A * NB], f32, tag="log")
                for p in range(A // 2):
                    nc.tensor.matmul(
                        log_ps[:, p * 8:(p + 1) * 8],
                        lhsT=W2blk[:, p * 6:(p + 1) * 6],
                        rhs=hid3[:, p * 8:(p + 1) * 8],
                        start=True, stop=True)
                stage6 = hw.tile([2 * ODIM, A * NB], f32, tag="stage6")
                nc.scalar.activation(stage6[:, :], log_ps[:, :], AF.Identity)
                logits = hw.tile([ODIM, A * NB], f32, tag="logits")
                # even assets: rows 0:3 at cols 8p..8p+4 (== a*4+b)
                nc.vector.tensor_copy(
                    _papp(logits, 0, 3, 0, [[8, 64], [1, 4]]),
                    _papp(stage6, 0, 3, 0, [[8, 64], [1, 4]]))
                # odd assets: rows 3:6 -> partition shift via DMA
                nc.sync.dma_start(
                    out=_papp(logits, 0, 3, 4, [[8, 64], [1, 4]]),
                    in_=_papp(stage6, 3, 6, 4, [[8, 64], [1, 4]]))
                nc.vector.tensor_tensor(out=logits[:, :], in0=logits[:, :],
                                        in1=b2exp[:, :], op=OP.add)
                nc.sync.dma_start(out=o_logits[:, :], in_=logits[:, :])

                # softmax over ODIM: transpose to (128, 4, 3), exp on eviction
                e_sb = hw.tile([128, NB * ODIM], f32, tag="e_sb")
                for c in range(NB):
                    pt = hps.tile([128, ODIM], f32, tag="sm", bufs=2)
                    nc.tensor.transpose(pt[:, :],
                                        logits[:, c * 128:(c + 1) * 128],
                                        identf[0:ODIM, 0:ODIM])
                    nc.scalar.activation(e_sb[:, c * ODIM:(c + 1) * ODIM],
                                         pt[:, :], AF.Exp)
                s_sb = hw.tile([128, NB], f32, tag="s_sb")
                for c in range(NB):
                    nc.vector.tensor_tensor(out=s_sb[:, c:c + 1],
                                            in0=e_sb[:, c * ODIM:c * ODIM + 1],
                                            in1=e_sb[:, c * ODIM + 1:c * ODIM + 2],
                                            op=OP.add)
                    nc.vector.tensor_tensor(out=s_sb[:, c:c + 1],
                                            in0=s_sb[:, c:c + 1],
                                            in1=e_sb[:, c * ODIM + 2:c * ODIM + 3],
                                            op=OP.add)
                r_sb = hw.tile([128, NB], f32, tag="r_sb")
                nc.vector.reciprocal(r_sb[:, :], s_sb[:, :])
                probs = hw.tile([128, NB * ODIM], f32, tag="probs")
                nc.vector.tensor_tensor(
                    out=probs[:, :], in0=e_sb[:, :],
                    in1=_ap(r_sb, 0, [[1, NB], [0, ODIM]]), op=OP.mult)
                nc.sync.dma_start(out=o_probs[:, :], in_=probs[:, :])

    return nc


def host_inputs(x, edge_index, W_emb, b_emb, conv_w, conv_b, bn_gamma, bn_beta,
                bn_mean, bn_var, gat_W, gat_a_src, gat_a_dst, ew_W1, ew_b1,
                ew_W2, ew_b2, head_W1, head_b1, head_W2, head_b2):
    """Per-core input dicts (host-side preprocessing)."""
    f = np.float32
    xs = np.asarray(x, f)[:, :, T - W:, :]                       # (B,N,15,64)
    xt = np.ascontiguousarray(np.transpose(xs, (3, 0, 1, 2)))    # (64,B,N,15)

    ei = np.asarray(edge_index)
    C = np.zeros((N, N), f)
    np.add.at(C, (ei[1].astype(np.int64), ei[0].astype(np.int64)), 1.0)
    CT = C.T.copy()                                              # [src, dst]

    cw = np.asarray(conv_w, f)                                   # (3,H,H,3)
    W_embf = np.asarray(W_emb, f)
    b_embf = np.asarray(b_emb, f)
    inv = np.asarray(bn_gamma, f) / np.sqrt(np.asarray(bn_var, f) + BN_EPS)
    sc_all = inv.T.copy()                                        # (H,3)
    cb_eff = np.asarray(conv_b, f).copy()
    cb_eff[0] = cb_eff[0] + cw[0].sum(axis=2) @ b_embf           # fold emb bias
    bi_all = ((cb_eff - np.asarray(bn_mean, f)) * inv
              + np.asarray(bn_beta, f)).T.copy()                 # (H,3)
    # conv1 taps folded with W_emb: (H,DIN) per tap; lhsT layout (DIN,H)
    c1wT = np.concatenate(
        [(cw[0, :, :, k] @ W_embf).T for k in range(3)], axis=1)  # (64,384)
    cwT = np.concatenate(
        [cw[i, :, :, k].T for i in (1, 2) for k in range(3)], axis=1)

    ew_W1 = np.asarray(ew_W1, f)
    gat_W = np.asarray(gat_W, f)
    hW1 = np.asarray(head_W1, f); hW2 = np.asarray(head_W2, f)
    hb1 = np.asarray(head_b1, f); hb2 = np.asarray(head_b2, f)

    # b1exp3[k-part, col=a*4+b]: rows 0:64 even-asset k, 64:128 odd-asset k
    b1exp3 = np.zeros((128, A * NB), f)
    for a in range(A):
        rows = slice(0, 64) if a % 2 == 0 else slice(64, 128)
        b1exp3[rows, a * NB:(a + 1) * NB] = hb1[a][:, None]
    # W2blk [128=(2a,64k), pair*6 + (2a,3o)] zero-padded block diagonal
    W2blk = np.zeros((H, (A // 2) * 2 * ODIM), f)
    for p in range(A // 2):
        W2blk[0:64, p * 6:p * 6 + 3] = hW2[2 * p].T           # (64k, 3o)
        W2blk[64:128, p * 6 + 3:p * 6 + 6] = hW2[2 * p + 1].T
    b2exp = np.repeat(hb2.T[:, :, None], NB, axis=2).reshape(ODIM, A * NB)

    bf = lambda a: np.ascontiguousarray(a).astype(BF)
    shared = {
        "c1wT": bf(c1wT),
        "cwT": bf(cwT),
        "sc_all": np.ascontiguousarray(sc_all),
        "bi_all": np.ascontiguousarray(bi_all),
        "W1aT": bf(ew_W1[:, :H].T),
        "W1bT": bf(ew_W1[:, H:].T),
        "b1f": np.asarray(ew_b1, f).reshape(H, 1),
        "w2b": bf(np.asarray(ew_W2, f).reshape(1, H).T),
        "b2ew": np.asarray(ew_b2, f).reshape(1, 1),
        "CAT": bf(CT[:128]),
        "CBT": bf(CT[128:]),
        "gWT": bf(np.concatenate([gat_W[i].T for i in range(3)], axis=1)),
        "asrcb": bf(np.stack([gat_W[i].T @ np.asarray(gat_a_src, f)[i, 0]
                              for i in range(3)], axis=1)),
        "adstb": bf(np.stack([gat_W[i].T @ np.asarray(gat_a_dst, f)[i, 0]
                              for i in range(3)], axis=1)),
        "hW1T": bf(np.concatenate([hW1[a].T for a in range(A)], axis=1)),
        "b1exp3": b1exp3,
        "W2blk": bf(W2blk),
        "b2exp": np.ascontiguousarray(b2exp),
    }
    in_maps = []
    for c in range(NC_CORES):
        m = dict(shared)
        m["xt"] = bf(xt[:, c * NB:(c + 1) * NB].reshape(DIN, NCOL * W))
        in_maps.append(m)
    return in_maps


_CACHE = {}


def kernel(**inputs):
    _apply_sync_split_patch()
    if "nc" not in _CACHE:
        _CACHE["nc"] = build_program()
    nc = _CACHE["nc"]
    in_maps = host_inputs(**inputs)
    res = run_bass_kernel_spmd(nc, in_maps, list(range(NC_CORES)), trace=False)
    logits = np.empty((B, A, ODIM), np.float32)
    probs = np.empty((B, A, ODIM), np.float32)
    for c in range(NC_CORES):
        lg = np.asarray(res.results[c]["logits"], np.float32)  # (3, A*NB)
        pr = np.asarray(res.results[c]["probs"], np.float32)   # (128, NB*3)
        logits[c * NB:(c + 1) * NB] = (
            lg.reshape(ODIM, A, NB).transpose(2, 1, 0))
        tmp = pr.reshape(128, NB, ODIM).transpose(1, 0, 2).reshape(A * NB, ODIM)
        probs[c * NB:(c + 1) * NB] = tmp.reshape(A, NB, ODIM).transpose(1, 0, 2)
    return logits, probs



# revision 18
# speedup vs baseline: 1.0889x; 1.0889x over previous
"""CrossAssetGNN forward on 8 Trainium2 cores, data-parallel over batch.

bf16 rewrite of the fp32 baseline (653us). Key structural changes:
- All matmuls/transposes run in bf16 (1 cyc/row vs 4 for fp32, single
  hardware pass instead of two, cheaper LDWEIGHTS).
- The node-embedding matmul is folded into conv1 on the host:
  conv1(emb(x)) = sum_k (C1k @ W_emb) @ x_shift_k, so the kernel starts
  straight from the 15-timestep x slice.
- Edge-weight MLP: R blocks are built src-major ((v,g,j) col order) by
  DVE+GpSimd halves (outer-sum via stride-0 broadcast APs), relu on the
  Act engine, then 520 bf16 matmuls (R-block stationary, w2 streaming)
  produce ew^T[src, (g,dst)] directly; the 2 aux-j rows collapse to four
  [1,260] row-matmuls plus 2 partition-shifting DMAs.
- GAT layers work on the TRANSPOSED attention matrix: alpha^T is built
  by PE rank-2 matmuls, the exp/count chain multiplies by C^T (host
  upload), so P^T (the aggregation stationary) appears with NO on-device
  transposes of P; row sums come from a ones-vector matmul; only the
  nf node-major -> feature-major transpose remains (2 per graph).
- Per-asset heads: 2-asset-packed stationaries with zero-padded
  block-diagonal W2, junk quadrants killed by the zero blocks.
"""
import json
import sys

sys.path.insert(0, "/opt/trn_rl_repo")

import numpy as np
import ml_dtypes
from contextlib import ExitStack

import concourse.bass as bass
import concourse.tile as tile
from concourse import masks, mybir
from concourse.bass_utils import run_bass_kernel_spmd

f32 = mybir.dt.float32
bf16 = mybir.dt.bfloat16
AF = mybir.ActivationFunctionType
OP = mybir.AluOpType
BF = ml_dtypes.bfloat16

B, A, AUX, T, DIN, H, ODIM = 32, 128, 2, 128, 64, 128, 3
N = A + AUX            # 130
NC_CORES = 8
NB = B // NC_CORES     # 4 graphs per core
W = 15                 # receptive field of the three causal convs
BN_EPS = 1e-5
NCOL = NB * N          # 520 node columns per core


# ---- walrus workaround: max 1 sync-wait command per instruction ------------
def _apply_sync_split_patch():
    if getattr(bass.Bass, "_sync_split_patched", False):
        return
    orig = bass.Bass.to_json_bytes

    def to_json_bytes(self, *a, **kw):
        m = json.loads(orig(self, *a, **kw))
        for f in m.get("functions", []):
            for blk in f.get("blocks", []):
                new = []
                for inst in blk.get("instructions", []):
                    si = inst.get("sync_info")
                    if (si and si.get("on_wait") and len(si["on_wait"]) > 1
                            and inst.get("engine") in
                            {"PE", "DVE", "Activation", "SP", "Pool"}):
                        waits = si["on_wait"]
                        for k, w in enumerate(waits[:-1]):
                            new.append({"engine": inst["engine"], "ins": [],
                                        "outs": [],
                                        "name": f"{inst['name']}-sw{k}",
                                        "opcode": "NoOp",
                                        "sync_info": {"on_update": [],
                                                      "on_wait": [w]}})
                        si["on_wait"] = waits[-1:]
                    new.append(inst)
                blk["instructions"] = new
        return json.dumps(m).encode()

    bass.Bass.to_json_bytes = to_json_bytes
    bass.Bass._sync_split_patched = True


def _ap(t, offset_elems, dims):
    """AP over tile t: partition dim + given free [step, count] dims."""
    return bass.AP(tensor=t.tensor, offset=t.offset + offset_elems,
                   ap=[list(t.ap[0])] + [list(d) for d in dims])


def _papp(t, p0, p1, offset_elems, dims):
    """AP with partition slice [p0:p1] + free dims."""
    base = t[p0:p1, 0:1]
    return bass.AP(tensor=base.tensor, offset=base.offset + offset_elems,
                   ap=[list(base.ap[0])] + [list(d) for d in dims])


def _chunks(total, step):
    return [(s, min(step, total - s)) for s in range(0, total, step)]


def build_program():
    nc = bass.Bass("TRN2", target_bir_lowering=False, num_devices=NC_CORES)

    din = {}

    def d_in(name, shape, dt=bf16):
        din[name] = nc.dram_tensor(name, list(shape), dt, kind="ExternalInput")
        return din[name]

    d_in("xt", [DIN, NCOL * W])
    d_in("c1wT", [DIN, 3 * H])          # conv1 taps folded with W_emb
    d_in("cwT", [H, 6 * H])             # conv2/conv3 taps
    d_in("sc_all", [H, 3], f32)
    d_in("bi_all", [H, 3], f32)
    d_in("W1aT", [H, H]); d_in("W1bT", [H, H])
    d_in("b1f", [H, 1], f32)
    d_in("w2b", [H, 1])
    d_in("b2ew", [1, 1], f32)
    d_in("CAT", [128, N]); d_in("CBT", [2, N])     # C^T rows
    d_in("gWT", [H, 3 * H])
    d_in("asrcb", [H, 3]); d_in("adstb", [H, 3])
    d_in("hW1T", [H, A * 64])
    d_in("b1exp3", [128, A * NB], f32)
    d_in("W2blk", [H, (A // 2) * 2 * ODIM])
    d_in("b2exp", [ODIM, A * NB], f32)

    o_logits = nc.dram_tensor("logits", [ODIM, A * NB], f32,
                              kind="ExternalOutput")
    o_probs = nc.dram_tensor("probs", [128, NB * ODIM], f32,
                             kind="ExternalOutput")

    with tile.TileContext(nc) as tc:
        with ExitStack() as top:
            const = top.enter_context(tc.tile_pool(name="const", bufs=1))
            persist = top.enter_context(tc.tile_pool(name="persist", bufs=1))
            front = top.enter_context(tc.tile_pool(name="front", bufs=1))

            # upload order: xt(g0) and the conv weights gate the pipeline
            # head, so they go before xt(g1..3) on the queues
            xT = front.tile([DIN, NCOL * W], bf16)

            def load(name, shape, dt=bf16):
                t = const.tile(list(shape), dt, name=f"c_{name}",
                               tag=f"c_{name}")
                nc.sync.dma_start(out=t, in_=din[name][:, :])
                return t

            nc.sync.dma_start(out=xT[:, 0:N * W], in_=din["xt"][:, 0:N * W])
            c1wT = load("c1wT", [DIN, 3 * H])
            sc_all = load("sc_all", [H, 3], f32)
            bi_all = load("bi_all", [H, 3], f32)
            cwT = load("cwT", [H, 6 * H])
            for g in range(1, NB):
                nc.sync.dma_start(
                    out=xT[:, g * N * W:(g + 1) * N * W],
                    in_=din["xt"][:, g * N * W:(g + 1) * N * W])
            W1aT = load("W1aT", [H, H]); W1bT = load("W1bT", [H, H])
            b1f = load("b1f", [H, 1], f32)
            w2b = load("w2b", [H, 1])
            CAT = load("CAT", [128, N]); CBT = load("CBT", [2, N])
            gWT = load("gWT", [H, 3 * H])
            asrcb = load("asrcb", [H, 3]); adstb = load("adstb", [H, 3])

            b2ap = din["b2ew"][:, :]
            b2col = const.tile([128, 1], f32)
            nc.sync.dma_start(out=b2col, in_=bass.AP(
                tensor=b2ap.tensor, offset=b2ap.offset, ap=[[0, 128], [1, 1]]))

            identb = const.tile([128, 128], bf16)
            masks.make_identity(nc, identb[:, :])
            identf = const.tile([128, 128], f32)
            masks.make_identity(nc, identf[:, :])
            alpha02 = const.tile([128, 1], f32)
            nc.vector.memset(alpha02[:, :], 0.2)
            ones128b = const.tile([128, 1], bf16)
            nc.vector.memset(ones128b[:, :], 1.0)
            ones_row = const.tile([1, NCOL], bf16)
            nc.vector.memset(ones_row[:, :], 1.0)

            feats = persist.tile([H, NCOL], bf16, name="feats")
            Ut = persist.tile([H, NCOL], bf16, name="Ut")
            Vt = persist.tile([H, NCOL], bf16, name="Vt")
            ewT_sb = persist.tile([128, NCOL], bf16, name="ewT")
            ewT2 = persist.tile([2, NCOL], bf16, name="ewT2")
            nfT_a = persist.tile([H, NCOL], bf16, name="nfT_a")
            nfT_b = persist.tile([H, NCOL], bf16, name="nfT_b")

            # ------- stages A+B fused: per-graph conv cascade + edge MLP ----
            # The conv chain is graph-local (conv2(g) reads only l1(g), etc),
            # so graph g's edge-MLP elementwise work (DVE/GpSimd) runs UNDER
            # the PE convs of graphs g+1.. .  R_g cols = vlp*256 + j*2 + t
            # (v = 2*vlp+t, j = src 0..127): the t in {0,1} inner pair gives
            # every DVE operand a stride-1 last AP dim -> 2x DVE mode
            # (0.53ns/elem) vs 1x for the plain broadcast outer-sum.
            RW = 65 * 256              # 16640 R cols per graph
            zero1 = const.tile([128, 1], bf16)
            nc.vector.memset(zero1[:, :], 0.0)
            with ExitStack() as sAB:
                psA = sAB.enter_context(
                    tc.tile_pool(name="psA", bufs=3, space="PSUM"))
                psU = sAB.enter_context(
                    tc.tile_pool(name="psU", bufs=1, space="PSUM"))
                psE = sAB.enter_context(
                    tc.tile_pool(name="psE", bufs=1, space="PSUM"))
                ewk = sAB.enter_context(tc.tile_pool(name="ewk", bufs=2))

                ewTPS4 = psE.tile([128, NB * N], f32, name="ewTPS4",
                                  tag="ewTPS4")


                # DVE fast-path note: concurrent GpSimd TensorTensor work
                # knocks DVE TensorTensor off its 2x packed mode, so stage B
                # elementwise runs on DVE+Act only. Adds use the proven-fast
                # (0,33),(33,32) vlp split.
                nblk = 65
                relu_dve = 28

                # conv1/conv2 chunks interleaved so PE always has independent
                # work while Act drains gelu evictions (breaks the psA
                # lockstep and lets the PE p-state ramp).
                l1 = ewk.tile([H, NCOL * 13], bf16, name="l1", tag="R3",
                              bufs=1)
                l2 = front.tile([H, NCOL * 9], bf16)

                def conv_chunk(src_, taps, tap0, in_len, out_len, dil, li,
                               dst, b0, nb):
                    sv = src_.rearrange("p (blk t) -> p blk t", t=in_len)
                    pe = psA.tile([128, 512], f32, tag="pe", name="pe")
                    w_cols = nb * out_len
                    for k in range(3):
                        rhs = sv[:, b0:b0 + nb, k * dil:k * dil + out_len]
                        nc.tensor.matmul(
                            pe[:, :w_cols],
                            lhsT=taps[:, (tap0 + k) * H:(tap0 + k + 1) * H],
                            rhs=rhs, start=(k == 0), stop=(k == 2))
                    nc.scalar.activation(
                        dst[:, b0 * out_len:b0 * out_len + w_cols],
                        pe[:, :w_cols], AF.Gelu,
                        bias=bi_all[:, li:li + 1], scale=sc_all[:, li:li + 1])

                c1chunks = _chunks(NCOL, 39)
                c2chunks = _chunks(NCOL, 56)
                c2i = 0
                for k1, (b0, nb) in enumerate(c1chunks):
                    conv_chunk(xT, c1wT, 0, W, 13, 1, 0, l1, b0, nb)
                    # emit any conv2 chunk whose l1 inputs are complete
                    done = 39 * (k1 + 1)
                    while (c2i < len(c2chunks)
                           and c2chunks[c2i][0] + c2chunks[c2i][1] <= done):
                        cb, cn = c2chunks[c2i]
                        conv_chunk(l1, cwT, 0, 13, 9, 2, 1, l2, cb, cn)
                        c2i += 1
                while c2i < len(c2chunks):
                    cb, cn = c2chunks[c2i]
                    conv_chunk(l1, cwT, 0, 13, 9, 2, 1, l2, cb, cn)
                    c2i += 1

                # conv3 + U/V for every graph first: keeps the Act FIFO free
                # of stage-B relu before the V evictions (which gate the DVE
                # adds of later graphs)
                l2v = l2.rearrange("p (blk t) -> p blk t", t=9)
                for g in range(NB):
                    gb = g * N
                    pe3 = psA.tile([128, 512], f32, tag="pe", name="pe3")
                    for k in range(3):
                        nc.tensor.matmul(
                            pe3[:, :N],
                            lhsT=cwT[:, (3 + k) * H:(4 + k) * H],
                            rhs=l2v[:, gb:gb + N, k * 4:k * 4 + 1],
                            start=(k == 0), stop=(k == 2))
                    nc.scalar.activation(feats[:, gb:gb + N], pe3[:, :N],
                                         AF.Gelu, bias=bi_all[:, 2:3],
                                         scale=sc_all[:, 2:3])
                    pu = psU.tile([128, N], f32, tag="uv")
                    nc.tensor.matmul(pu[:, :], lhsT=W1aT[:, :],
                                     rhs=feats[:, gb:gb + N], start=True,
                                     stop=True)
                    nc.vector.tensor_copy(Ut[:, gb:gb + N], pu[:, :])
                    pv = psU.tile([128, N], f32, tag="uv")
                    nc.tensor.matmul(pv[:, :], lhsT=W1bT[:, :],
                                     rhs=feats[:, gb:gb + N], start=True,
                                     stop=True)
                    nc.scalar.activation(Vt[:, gb:gb + N], pv[:, :],
                                         AF.Identity, bias=b1f[:, :])

                # per-graph pair-interleaved outer-sum + relu (DVE + Act)
                Rt = []
                for g in range(NB):
                    gb = g * N
                    R = ewk.tile([128, RW], bf16, name=f"R{g}",
                                 tag=f"R{g}", bufs=1)
                    Rt.append(R)
                    Udup = ewk.tile([128, 256], bf16, tag="Udup", bufs=2)
                    nc.vector.tensor_copy(
                        _ap(Udup, 0, [[2, 128], [1, 2]]),
                        _ap(Ut, gb, [[1, 128], [0, 2]]))
                    for h0, hn in ((0, 33), (33, 32)):
                        nc.vector.tensor_tensor(
                            out=_ap(R, h0 * 256, [[256, hn], [1, 256]]),
                            in0=_ap(Vt, gb + 2 * h0, [[2, hn], [0, 128],
                                                      [1, 2]]),
                            in1=_ap(Udup, 0, [[0, hn], [1, 256]]),
                            op=OP.add)
                    nc.vector.tensor_scalar_max(
                        _ap(R, 0, [[256, relu_dve], [1, 256]]),
                        _ap(R, 0, [[256, relu_dve], [1, 256]]), 0.0)
                    for r0, rn in _chunks(nblk - relu_dve, 19):
                        nc.scalar.activation(
                            R[:, (relu_dve + r0) * 256:
                              (relu_dve + r0 + rn) * 256],
                            R[:, (relu_dve + r0) * 256:
                              (relu_dve + r0 + rn) * 256], AF.Relu)

                # aux-j rows (src 128..129): cols (j2, g, v)
                Raux = ewk.tile([128, 2 * NCOL], bf16, tag="Raux")
                for j2 in range(2):
                    nc.vector.tensor_tensor(
                        out=_ap(Raux, j2 * NCOL, [[N, NB], [1, N]]),
                        in0=_ap(Ut, 128 + j2, [[N, NB], [0, N]]),
                        in1=_ap(Vt, 0, [[N, NB], [1, N]]),
                        op=OP.add)
                nc.scalar.activation(Raux[:, :], Raux[:, :], AF.Relu)
                ewstage = ewk.tile([1, 2 * NCOL], bf16, tag="ewstage")
                for c in range(4):
                    pax = psU.tile([1, 260], f32, tag="aux")
                    nc.tensor.matmul(pax[0:1, :], lhsT=w2b[:, :],
                                     rhs=Raux[:, c * 260:(c + 1) * 260],
                                     start=True, stop=True)
                    nc.scalar.activation(ewstage[0:1, c * 260:(c + 1) * 260],
                                         pax[0:1, :], AF.Sigmoid,
                                         bias=b2col[0:1, :])
                nc.sync.dma_start(out=ewT2[0:1, :], in_=ewstage[0:1, 0:NCOL])
                nc.sync.dma_start(out=ewT2[1:2, :],
                                  in_=ewstage[0:1, NCOL:2 * NCOL])

                # reduce (strided 1-col matmuls, LDWEIGHTS-free) + sigmoid
                for g in range(NB):
                    R = Rt[g]
                    for vlp in range(nblk):
                        for t in range(2):
                            v = 2 * vlp + t
                            nc.tensor.matmul(
                                ewTPS4[:, g * N + v:g * N + v + 1],
                                lhsT=_ap(R, vlp * 256 + t, [[2, 128]]),
                                rhs=w2b[:, :], start=True, stop=True)
                    nc.scalar.activation(
                        ewT_sb[:, g * N:(g + 1) * N],
                        ewTPS4[:, g * N:(g + 1) * N], AF.Sigmoid,
                        bias=b2col[:, :])

            # late-load the bulky head weights (needed only in stage D)
            hW1T = load("hW1T", [H, A * 64])
            b1exp3 = load("b1exp3", [128, A * NB], f32)
            W2blk = load("W2blk", [H, (A // 2) * 2 * ODIM])
            b2exp = load("b2exp", [ODIM, A * NB], f32)

            # ---------------- stage C: 3 GAT layers (transposed P) ----------
            nfT_cur = feats
            with ExitStack() as sC:
                gw = sC.enter_context(tc.tile_pool(name="gw", bufs=2))
                gps = sC.enter_context(
                    tc.tile_pool(name="gps", bufs=1, space="PSUM"))

                # AS2 rows: (as, ones); AD2 rows: (ones, ad) — const rows
                # preset once, per-layer rows written below
                AS2 = persist.tile([2, NCOL], bf16, name="AS2")
                nc.sync.dma_start(out=AS2[1:2, :], in_=ones_row[0:1, :])
                AD2 = persist.tile([2, NCOL], bf16, name="AD2")
                nc.vector.tensor_copy(AD2[0:1, :], ones_row[0:1, :])

                for li in range(3):
                    gW = gWT[:, li * H:(li + 1) * H]
                    nfT_next = nfT_a if li % 2 == 0 else nfT_b

                    # as/ad rows directly from nfT via host-folded W^T a
                    ad_stage = gw.tile([1, NCOL], bf16, tag="ad_stage")
                    for s, ln in _chunks(NCOL, 512):
                        pr = gps.tile([128, 512], f32, tag="big", bufs=1)
                        nc.tensor.matmul(pr[0:1, :ln],
                                         lhsT=asrcb[:, li:li + 1],
                                         rhs=nfT_cur[:, s:s + ln],
                                         start=True, stop=True)
                        nc.vector.tensor_copy(AS2[0:1, s:s + ln],
                                              pr[0:1, :ln])
                        pr2 = gps.tile([128, 512], f32, tag="big", bufs=1)
                        nc.tensor.matmul(pr2[0:1, :ln],
                                         lhsT=adstb[:, li:li + 1],
                                         rhs=nfT_cur[:, s:s + ln],
                                         start=True, stop=True)
                        nc.vector.tensor_copy(ad_stage[0:1, s:s + ln],
                                              pr2[0:1, :ln])
                    nc.sync.dma_start(out=AD2[1:2, :], in_=ad_stage[0:1, :])

                    tT = gw.tile([128, NCOL], bf16, tag="tT")
                    tT2 = gw.tile([2, NCOL], bf16, tag="tT2")
                    for gp in range(2):
                        pac = gps.tile([128, 264], f32, tag="pa", bufs=1)
                        pac2 = gps.tile([2, 264], f32, tag="pa2", bufs=1)
                        for k in range(2):
                            g = gp * 2 + k
                            nc.tensor.matmul(pac[:, k * N:(k + 1) * N],
                                             lhsT=AS2[:, g * N:g * N + 128],
                                             rhs=AD2[:, g * N:(g + 1) * N],
                                             start=True, stop=True)
                            nc.tensor.matmul(pac2[0:2, k * N:(k + 1) * N],
                                             lhsT=AS2[:, g * N + 128:(g + 1) * N],
                                             rhs=AD2[:, g * N:(g + 1) * N],
                                             start=True, stop=True)
                        nc.scalar.activation(tT[:, gp * 2 * N:(gp + 1) * 2 * N],
                                             pac[:, 0:2 * N], AF.Prelu,
                                             alpha=alpha02[:, :])
                        nc.scalar.activation(tT2[0:2, gp * 2 * N:(gp + 1) * 2 * N],
                                             pac2[0:2, 0:2 * N], AF.Prelu,
                                             alpha=alpha02[0:2, :])

                    zT = gw.tile([128, NCOL], bf16, tag="zT")
                    nc.vector.tensor_tensor(out=zT[:, :], in0=tT[:, :],
                                            in1=ewT_sb[:, :], op=OP.mult)
                    zT2 = gw.tile([2, NCOL], bf16, tag="zT2")
                    nc.vector.tensor_tensor(out=zT2[:, :], in0=tT2[:, :],
                                            in1=ewT2[:, :], op=OP.mult)
                    eT = gw.tile([128, NCOL], bf16, tag="eT")
                    nc.scalar.activation(eT[:, :], zT[:, :], AF.Exp)
                    eT2 = gw.tile([2, NCOL], bf16, tag="eT2")
                    nc.scalar.activation(eT2[:, :], zT2[:, :], AF.Exp)
                    PT = gw.tile([128, NCOL], bf16, tag="PT")
                    nc.vector.tensor_tensor(
                        out=PT[:, :], in0=eT[:, :],
                        in1=_ap(CAT, 0, [[0, NB], [1, N]]), op=OP.mult)
                    PT2 = gw.tile([2, NCOL], bf16, tag="PT2")
                    nc.vector.tensor_tensor(
                        out=PT2[:, :], in0=eT2[:, :],
                        in1=_ap(CBT, 0, [[0, NB], [1, N]]), op=OP.mult)

                    # per-dst row sums as psum columns (no transposes needed)
                    sumsPS = gps.tile([128, 8], f32, tag="sums", bufs=1)
                    for g in range(NB):
                        nc.tensor.matmul(sumsPS[:, g:g + 1],
                                         lhsT=PT[:, g * N:g * N + 128],
                                         rhs=ones128b[:, :],
                                         start=True, stop=False)
                        nc.tensor.matmul(sumsPS[:, g:g + 1],
                                         lhsT=PT2[:, g * N:g * N + 128],
                                         rhs=ones128b[0:2, :],
                                         start=False, stop=True)
                        if li < 2:
                            nc.tensor.matmul(
                                sumsPS[0:2, 4 + g:5 + g],
                                lhsT=PT[:, g * N + 128:(g + 1) * N],
                                rhs=ones128b[:, :], start=True, stop=False)
                            nc.tensor.matmul(
                                sumsPS[0:2, 4 + g:5 + g],
                                lhsT=PT2[:, g * N + 128:(g + 1) * N],
                                rhs=ones128b[0:2, :], start=False, stop=True)
                    rAe = gw.tile([128, NB], f32, tag="rAe")
                    nc.vector.tensor_scalar_add(rAe[:, :], sumsPS[:, 0:4],
                                                1e-8)
                    rA = gw.tile([128, NB], f32, tag="rA")
                    nc.vector.reciprocal(rA[:, :], rAe[:, :])
                    rexp = gw.tile([128, NB * H], bf16, tag="rexp")
                    nc.vector.tensor_copy(rexp[:, :],
                                          _ap(rA, 0, [[1, NB], [0, H]]))
                    if li < 2:
                        rBe = gw.tile([2, NB], f32, tag="rBe")
                        nc.vector.tensor_scalar_add(rBe[:, :],
                                                    sumsPS[0:2, 4:8], 1e-8)
                        rB = gw.tile([2, NB], f32, tag="rB")
                        nc.vector.reciprocal(rB[:, :], rBe[:, :])
                        rexp2 = gw.tile([2, NB * H], bf16, tag="rexp2")
                        nc.vector.tensor_copy(rexp2[:, :],
                                              _ap(rB, 0, [[1, NB], [0, H]]))

                    poPS = gps.tile([128, NB * H], f32, tag="po", bufs=1)
                    if li < 2:
                        poPS2 = gps.tile([2, NB * H], f32, tag="po2", bufs=1)
                    for g in range(NB):
                        sq = gps.tile([128, 256], f32, tag="sq", bufs=1)
                        nc.tensor.matmul(sq[:, 0:H],
                                         lhsT=nfT_cur[:, g * N:g * N + 128],
                                         rhs=gW, start=True, stop=True)
                        hpA = gw.tile([128, H], bf16, tag="hpA")
                        nc.vector.tensor_copy(hpA[:, :], sq[:, 0:H])
                        nc.tensor.matmul(sq[0:2, H:2 * H],
                                         lhsT=nfT_cur[:, g * N + 128:(g + 1) * N],
                                         rhs=gW, start=True, stop=True)
                        hpB = gw.tile([2, H], bf16, tag="hpB")
                        nc.vector.tensor_copy(hpB[:, :], sq[0:2, H:2 * H])

                        nc.tensor.matmul(poPS[:, g * H:(g + 1) * H],
                                         lhsT=PT[:, g * N:g * N + 128],
                                         rhs=hpA[:, :], start=True, stop=False)
                        nc.tensor.matmul(poPS[:, g * H:(g + 1) * H],
                                         lhsT=PT2[:, g * N:g * N + 128],
                                         rhs=hpB[:, :], start=False, stop=True)
                        if li < 2:
                            nc.tensor.matmul(
                                poPS2[0:2, g * H:(g + 1) * H],
                                lhsT=PT[:, g * N + 128:(g + 1) * N],
                                rhs=hpA[:, :], start=True, stop=False)
                            nc.tensor.matmul(
                                poPS2[0:2, g * H:(g + 1) * H],
                                lhsT=PT2[:, g * N + 128:(g + 1) * N],
                                rhs=hpB[:, :], start=False, stop=True)

                    # batched elu over all 4 graphs: elu(po*r) with r>0
                    pos_all = gw.tile([128, NB * H], bf16, tag="pos_all")
                    nc.scalar.activation(pos_all[:, :], poPS[:, :], AF.Relu)
                    posr = gw.tile([128, NB * H], bf16, tag="posr")
                    nc.vector.tensor_tensor(out=posr[:, :], in0=pos_all[:, :],
                                            in1=rexp[:, :], op=OP.mult)
                    m_all = gw.tile([128, NB * H], bf16, tag="m_all")
                    nc.vector.tensor_scalar_min(m_all[:, :], poPS[:, :], 0.0)
                    mr = gw.tile([128, NB * H], bf16, tag="mr")
                    nc.vector.tensor_tensor(out=mr[:, :], in0=m_all[:, :],
                                            in1=rexp[:, :], op=OP.mult)
                    exm = gw.tile([128, NB * H], bf16, tag="exm")
                    nc.scalar.activation(exm[:, :], mr[:, :], AF.Exp)
                    nf_nm = gw.tile([128, NB * H], bf16, tag="nf_nm")
                    nc.vector.scalar_tensor_tensor(
                        out=nf_nm[:, :], in0=exm[:, :], scalar=1.0,
                        in1=posr[:, :], op0=OP.subtract, op1=OP.add)
                    if li < 2:
                        pos2 = gw.tile([2, NB * H], bf16, tag="pos2")
                        nc.scalar.activation(pos2[:, :], poPS2[:, :], AF.Relu)
                        posr2 = gw.tile([2, NB * H], bf16, tag="posr2")
                        nc.vector.tensor_tensor(out=posr2[:, :],
                                                in0=pos2[:, :],
                                                in1=rexp2[:, :], op=OP.mult)
                        m2 = gw.tile([2, NB * H], bf16, tag="m2")
                        nc.vector.tensor_scalar_min(m2[:, :], poPS2[:, :], 0.0)
                        mr2 = gw.tile([2, NB * H], bf16, tag="mr2")
                        nc.vector.tensor_tensor(out=mr2[:, :], in0=m2[:, :],
                                                in1=rexp2[:, :], op=OP.mult)
                        exm2 = gw.tile([2, NB * H], bf16, tag="exm2")
                        nc.scalar.activation(exm2[:, :], mr2[:, :], AF.Exp)
                        nf_nm2 = gw.tile([2, NB * H], bf16, tag="nf_nm2")
                        nc.vector.scalar_tensor_tensor(
                            out=nf_nm2[:, :], in0=exm2[:, :], scalar=1.0,
                            in1=posr2[:, :], op0=OP.subtract, op1=OP.add)

                    for g in range(NB):
                        ptb = gps.tile([128, 130], bf16, tag="tb", bufs=1)
                        nc.tensor.transpose(ptb[:, 0:128],
                                            nf_nm[:, g * H:(g + 1) * H],
                                            identb[:, :])
                        nc.vector.tensor_copy(nfT_next[:, g * N:g * N + 128],
                                              ptb[:, 0:128])
                        if li < 2:
                            nc.tensor.transpose(ptb[:, 128:130],
                                                nf_nm2[:, g * H:(g + 1) * H],
                                                identb[0:2, 0:2])
                            nc.vector.tensor_copy(
                                nfT_next[:, g * N + 128:(g + 1) * N],
                                ptb[:, 128:130])
                    nfT_cur = nfT_next

            # ---------------- stage D: packed per-asset heads + softmax -----
            with ExitStack() as sD:
                hw = sD.enter_context(tc.tile_pool(name="hw", bufs=1))
                hps = sD.enter_context(
                    tc.tile_pool(name="hps", bufs=1, space="PSUM"))

                hid_ps = hps.tile([128, A * NB], f32, tag="hid")
                for p in range(A // 2):
                    nc.tensor.matmul(
                        hid_ps[:, p * 8:(p + 1) * 8],
                        lhsT=hW1T[:, p * 128:(p + 1) * 128],
                        rhs=_ap(nfT_cur, 2 * p, [[1, 2], [N, NB]]),
                        start=True, stop=True)
                hid_t = hw.tile([128, A * NB], bf16, tag="hid_t")
                nc.vector.tensor_tensor(out=hid_t[:, :], in0=hid_ps[:, :],
                                        in1=b1exp3[:, :], op=OP.add)
                hid3 = hw.tile([128, A * NB], bf16, tag="hid3")
                nc.scalar.activation(hid3[:, :], hid_t[:, :], AF.Relu)

                log_ps = hps.tile([2 * ODIM, A * NB], f32, tag="log")
                for p in range(A // 2):
                    nc.tensor.matmul(
                        log_ps[:, p * 8:(p + 1) * 8],
                        lhsT=W2blk[:, p * 6:(p + 1) * 6],
                        rhs=hid3[:, p * 8:(p + 1) * 8],
                        start=True, stop=True)
                stage6 = hw.tile([2 * ODIM, A * NB], f32, tag="stage6")
                nc.scalar.activation(stage6[:, :], log_ps[:, :], AF.Identity)
                logits = hw.tile([ODIM, A * NB], f32, tag="logits")
                # even assets: rows 0:3 at cols 8p..8p+4 (== a*4+b)
                nc.vector.tensor_copy(
                    _papp(logits, 0, 3, 0, [[8, 64], [1, 4]]),
                    _papp(stage6, 0, 3, 0, [[8, 64], [1, 4]]))
                # odd assets: rows 3:6 -> partition shift via DMA
                nc.sync.dma_start(
                    out=_papp(logits, 0, 3, 4, [[8, 64], [1, 4]]),
                    in_=_papp(stage6, 3, 6, 4, [[8, 64], [1, 4]]))
                nc.vector.tensor_tensor(out=logits[:, :], in0=logits[:, :],
                                        in1=b2exp[:, :], op=OP.add)
                nc.sync.dma_start(out=o_logits[:, :], in_=logits[:, :])

                # softmax over ODIM: transpose to (128, 4, 3), exp on eviction
                e_sb = hw.tile([128, NB * ODIM], f32, tag="e_sb")
                for c in range(NB):
                    pt = hps.tile([128, ODIM], f32, tag="sm", bufs=2)
                    nc.tensor.transpose(pt[:, :],
                                        logits[:, c * 128:(c + 1) * 128],
                                        identf[0:ODIM, 0:ODIM])
                    nc.scalar.activation(e_sb[:, c * ODIM:(c + 1) * ODIM],
                                         pt[:, :], AF.Exp)
                s_sb = hw.tile([128, NB], f32, tag="s_sb")
                for c in range(NB):
                    nc.vector.tensor_tensor(out=s_sb[:, c:c + 1],
                                            in0=e_sb[:, c * ODIM:c * ODIM + 1],
                                            in1=e_sb[:, c * ODIM + 1:c * ODIM + 2],
                                            op=OP.add)
                    nc.vector.tensor_tensor(out=s_sb[:, c:c + 1],
                                            in0=s_sb[:, c:c + 1],
                                            in1=e_sb[:, c * ODIM + 2:c * ODIM + 3],
                                            op=OP.add)
                r_sb = hw.tile([128, NB], f32, tag="r_sb")
                nc.vector.reciprocal(r_sb[:, :], s_sb[:, :])
                probs = hw.tile([128, NB * ODIM], f32, tag="probs")
                nc.vector.tensor_tensor(
                    out=probs[:, :], in0=e_sb[:, :],
                    in1=_ap(r_sb, 0, [[1, NB], [0, ODIM]]), op=OP.mult)
                nc.sync.dma_start(out=o_probs[:, :], in_=probs[:, :])

    return nc


def host_inputs(x, edge_index, W_emb, b_emb, conv_w, conv_b, bn_gamma, bn_beta,
                bn_mean, bn_var, gat_W, gat_a_src, gat_a_dst, ew_W1, ew_b1,
                ew_W2, ew_b2, head_W1, head_b1, head_W2, head_b2):
    """Per-core input dicts (host-side preprocessing)."""
    f = np.float32
    xs = np.asarray(x, f)[:, :, T - W:, :]                       # (B,N,15,64)
    xt = np.ascontiguousarray(np.transpose(xs, (3, 0, 1, 2)))    # (64,B,N,15)

    ei = np.asarray(edge_index)
    C = np.zeros((N, N), f)
    np.add.at(C, (ei[1].astype(np.int64), ei[0].astype(np.int64)), 1.0)
    CT = C.T.copy()                                              # [src, dst]

    cw = np.asarray(conv_w, f)                                   # (3,H,H,3)
    W_embf = np.asarray(W_emb, f)
    b_embf = np.asarray(b_emb, f)
    inv = np.asarray(bn_gamma, f) / np.sqrt(np.asarray(bn_var, f) + BN_EPS)
    sc_all = inv.T.copy()                                        # (H,3)
    cb_eff = np.asarray(conv_b, f).copy()
    cb_eff[0] = cb_eff[0] + cw[0].sum(axis=2) @ b_embf           # fold emb bias
    bi_all = ((cb_eff - np.asarray(bn_mean, f)) * inv
              + np.asarray(bn_beta, f)).T.copy()                 # (H,3)
    # conv1 taps folded with W_emb: (H,DIN) per tap; lhsT layout (DIN,H)
    c1wT = np.concatenate(
        [(cw[0, :, :, k] @ W_embf).T for k in range(3)], axis=1)  # (64,384)
    cwT = np.concatenate(
        [cw[i, :, :, k].T for i in (1, 2) for k in range(3)], axis=1)

    ew_W1 = np.asarray(ew_W1, f)
    gat_W = np.asarray(gat_W, f)
    hW1 = np.asarray(head_W1, f); hW2 = np.asarray(head_W2, f)
    hb1 = np.asarray(head_b1, f); hb2 = np.asarray(head_b2, f)

    # b1exp3[k-part, col=a*4+b]: rows 0:64 even-asset k, 64:128 odd-asset k
    b1exp3 = np.zeros((128, A * NB), f)
    for a in range(A):
        rows = slice(0, 64) if a % 2 == 0 else slice(64, 128)
        b1exp3[rows, a * NB:(a + 1) * NB] = hb1[a][:, None]
    # W2blk [128=(2a,64k), pair*6 + (2a,3o)] zero-padded block diagonal
    W2blk = np.zeros((H, (A // 2) * 2 * ODIM), f)
    for p in range(A // 2):
        W2blk[0:64, p * 6:p * 6 + 3] = hW2[2 * p].T           # (64k, 3o)
        W2blk[64:128, p * 6 + 3:p * 6 + 6] = hW2[2 * p + 1].T
    b2exp = np.repeat(hb2.T[:, :, None], NB, axis=2).reshape(ODIM, A * NB)

    bf = lambda a: np.ascontiguousarray(a).astype(BF)
    shared = {
        "c1wT": bf(c1wT),
        "cwT": bf(cwT),
        "sc_all": np.ascontiguousarray(sc_all),
        "bi_all": np.ascontiguousarray(bi_all),
        "W1aT": bf(ew_W1[:, :H].T),
        "W1bT": bf(ew_W1[:, H:].T),
        "b1f": np.asarray(ew_b1, f).reshape(H, 1),
        "w2b": bf(np.asarray(ew_W2, f).reshape(1, H).T),
        "b2ew": np.asarray(ew_b2, f).reshape(1, 1),
        "CAT": bf(CT[:128]),
        "CBT": bf(CT[128:]),
        "gWT": bf(np.concatenate([gat_W[i].T for i in range(3)], axis=1)),
        "asrcb": bf(np.stack([gat_W[i].T @ np.asarray(gat_a_src, f)[i, 0]
                              for i in range(3)], axis=1)),
        "adstb": bf(np.stack([gat_W[i].T @ np.asarray(gat_a_dst, f)[i, 0]
                              for i in range(3)], axis=1)),
        "hW1T": bf(np.concatenate([hW1[a].T for a in range(A)], axis=1)),
        "b1exp3": b1exp3,
        "W2blk": bf(W2blk),
        "b2exp": np.ascontiguousarray(b2exp),
    }
    in_maps = []
    for c in range(NC_CORES):
        m = dict(shared)
        m["xt"] = bf(xt[:, c * NB:(c + 1) * NB].reshape(DIN, NCOL * W))
        in_maps.append(m)
    return in_maps


_CACHE = {}


def kernel(**inputs):
    _apply_sync_split_patch()
    if "nc" not in _CACHE:
        _CACHE["nc"] = build_program()
    nc = _CACHE["nc"]
    in_maps = host_inputs(**inputs)
    res = run_bass_kernel_spmd(nc, in_maps, list(range(NC_CORES)), trace=False)
    logits = np.empty((B, A, ODIM), np.float32)
    probs = np.empty((B, A, ODIM), np.float32)
    for c in range(NC_CORES):
        lg = np.asarray(res.results[c]["logits"], np.float32)  # (3, A*NB)
        pr = np.asarray(res.results[c]["probs"], np.float32)   # (128, NB*3)
        logits[c * NB:(c + 1) * NB] = (
            lg.reshape(ODIM, A, NB).transpose(2, 1, 0))
        tmp = pr.reshape(128, NB, ODIM).transpose(1, 0, 2).reshape(A * NB, ODIM)
        probs[c * NB:(c + 1) * NB] = tmp.reshape(A, NB, ODIM).transpose(1, 0, 2)
    return logits, probs

